# revision 12
# baseline (speedup 1.0000x reference)
"""Tensor-parallel causal-self-attention (full-attention) Bass kernel for TRN2.

Sharding: 16 heads over 8 cores (2 heads/core). Each core computes its heads'
QKV projections, rope, full attention, and its partial output projection
(rows of Wo for its heads); the host sums the 8 partial outputs (the
all-reduce of the tensor-parallel pattern, done at gather time).

Per-core layouts (everything "transposed", tokens on the free axis). All
matmul operands are bf16 (same PE rate as f32r, half the DMA/SBUF), psum f32:
  xT      [D=2048, B*T=4096]   x transposed (host-prepped, bf16), replicated
  wq/wk   [2048, 256]          head-column shard; within each head the 128
                               columns are permuted evens-then-odds so rope
                               pairs become contiguous partition halves
  wv      [2048, 256]          natural column shard
  wo      [256, 2048]          natural row shard
  cs1     [128, 2048]          [cos.T ; sin.T] stacked (64+64 partitions), f32
  cs2     [128, 2048]          [sin.T ; cos.T]
  ones    [128, 128]           all-ones (softmax denominator broadcast matmul)

Initial DMAs are issued fine-grained in priority order (wq, x half-batch 0,
wk, wv, cs, wo) so the first matmul starts early; x is loaded in half-batch
tiles (1024 tokens -> 2KB DMA lines) chunked 4-ways for subtile overlap.

Pipeline per batch b in {0,1}:
  A) qT/kT = W.T @ xT (per head, 512-token psum blocks, N=512 matmuls);
     rope via partition-half realign DMAs; v = xT.T @ wv (natural
     [token,256] tiles).  The previous batch's last output-projection block
     is interleaved one tile per rope group.
  B) per (head, i-block of 512): score PAIRS into a 2-bank psum tile
     ([128,1024]); ONE exp ACT per pair (psum->sbuf bf16); oT += v.T @ e.
     Softmax denominator: e-pairs are accumulated into two partial sums
     (DVE takes even pairs, gpsimd odd pairs, both bf16 SBUF) and a single
     ones-matmul pair broadcasts the partition-sum — replaces 16 full
     denominator matmuls per (h,ib) with 2 (saves ~50us of PE streaming).
     oT_norm = op * recip_approx(denom).
  C) y[t,d] = sum_h oT_h.T @ wo_h. The i-block's projection is deferred one
     i-block and its 16 psum tiles are emitted round-robin between score/PV
     pairs, so the PE (not the exp ACT) paces the attention inner loop.
     psum->sbuf staging copies split scalar/vector; DMA out (partial sums).
"""

import sys

sys.path.insert(0, "/opt/trn_rl_repo")

import numpy as np
import ml_dtypes

import concourse.bass as bass
import concourse.mybir as mybir
import concourse.tile as tile
from concourse import bacc
from concourse.bass_utils import run_bass_kernel_spmd

B, T, D = 2, 2048, 2048
NH, HD = 16, 128
NCORES = 8
HPC = NH // NCORES          # heads per core = 2
CPC = HPC * HD              # proj columns per core = 256
BT = B * T                  # 4096 tokens
P = 128
TBLK = 512                  # phase-A token block
NBLK = T // TBLK            # 4 blocks per batch
HB = 2 * TBLK               # x half-batch tile token width (1024)
DC = D // P                 # 16 contraction chunks
IBLK = 512                  # phase-B query block
NIB = T // IBLK             # 4 i-blocks per batch
NJT = T // P                # 16 key tiles per batch
NPR = NJT // 2              # 8 key-tile pairs
NYT = (IBLK // P) * (D // IBLK)  # outproj psum tiles per i-block = 16
SCALE = 1.0 / float(np.sqrt(HD))

f32 = mybir.dt.float32
bf16 = mybir.dt.bfloat16

_compiled = {}

# exposed for test.py
last_results = None


def _build():
    nc = bacc.Bacc("TRN2", target_bir_lowering=False, debug=False)

    # all weights and x arrive host-prepacked as their exact SBUF images so
    # every DMA is contiguous per partition (8-32KB lines, full bandwidth)
    xT_d = nc.dram_tensor("xT", [(BT // HB) * P, DC * HB], bf16,
                          kind="ExternalInput").ap()
    wq_d = nc.dram_tensor("wq", [P, DC * CPC], bf16, kind="ExternalInput").ap()
    wk_d = nc.dram_tensor("wk", [P, DC * CPC], bf16, kind="ExternalInput").ap()
    wv_d = nc.dram_tensor("wv", [P, DC * CPC], bf16, kind="ExternalInput").ap()
    wo_d = nc.dram_tensor("wo", [P, HPC * D], bf16, kind="ExternalInput").ap()
    cs1_d = nc.dram_tensor("cs1", [P, T], f32, kind="ExternalInput").ap()
    cs2_d = nc.dram_tensor("cs2", [P, T], f32, kind="ExternalInput").ap()
    ones_d = nc.dram_tensor("ones", [P, P], bf16, kind="ExternalInput").ap()
    y_d = nc.dram_tensor("y", [BT, D], f32, kind="ExternalOutput").ap()

    with tile.TileContext(nc) as tc:
        _emit(nc, tc, xT_d, wq_d, wk_d, wv_d, wo_d, cs1_d, cs2_d, ones_d, y_d)
    nc.compile()
    return nc


def _emit(nc, tc, xT_d, wq_d, wk_d, wv_d, wo_d, cs1_d, cs2_d, ones_d, y_d):
    from contextlib import ExitStack

    Exp = mybir.ActivationFunctionType.Exp
    mult = mybir.AluOpType.mult
    add = mybir.AluOpType.add
    sub = mybir.AluOpType.subtract

    with ExitStack() as ctx:
        const = ctx.enter_context(tc.tile_pool(name="const", bufs=1))
        state = ctx.enter_context(tc.tile_pool(name="state", bufs=1))
        xpool = ctx.enter_context(tc.tile_pool(name="xa", bufs=2))
        rpool = ctx.enter_context(tc.tile_pool(name="ra", bufs=4))
        epool = ctx.enter_context(tc.tile_pool(name="e", bufs=3))
        accpool = ctx.enter_context(tc.tile_pool(name="ac", bufs=2))
        rcpool = ctx.enter_context(tc.tile_pool(name="rc", bufs=2))
        ypool = ctx.enter_context(tc.tile_pool(name="yb", bufs=4))
        yps = ctx.enter_context(tc.tile_pool(name="y_ps", bufs=2, space="PSUM"))

        wq_sb = const.tile([P, DC * CPC], bf16, tag="wq")
        wk_sb = const.tile([P, DC * CPC], bf16, tag="wk")
        wv_sb = const.tile([P, DC * CPC], bf16, tag="wv")
        wo_sb = const.tile([P, HPC * D], bf16, tag="wo")
        cs1_sb = const.tile([P, T], f32, tag="cs1")
        cs2_sb = const.tile([P, T], f32, tag="cs2")
        ones_sb = const.tile([P, P], bf16, tag="ones")

        qT_sb = state.tile([P, HPC * T], bf16, tag="qT")
        kT_sb = state.tile([P, HPC * T], bf16, tag="kT")
        v_sb = state.tile([P, NJT * CPC], bf16, tag="v")
        oT_sb = state.tile([P, HPC * T], bf16, tag="oT")

        def load_w(sb, dr, c0, c1):
            # dc-chunk [c0, c1) of a prepacked weight image (contiguous)
            nc.sync.dma_start(sb[:, c0 * CPC:c1 * CPC],
                              dr[:, c0 * CPC:c1 * CPC])

        xtiles = {}

        def load_xh(b, half):
            # half-batch tile (1024 tokens), prepacked: contiguous 8KB chunks
            hbi = b * 2 + half
            xt = xpool.tile([P, DC * HB], bf16, tag="x")
            for c0, c1 in ((0, 4), (4, 8), (8, 12), (12, 16)):
                nc.sync.dma_start(
                    xt[:, c0 * HB:c1 * HB],
                    xT_d[hbi * P:(hbi + 1) * P, c0 * HB:c1 * HB])
            xtiles[(b, half)] = xt

        # priority-ordered startup: what the first matmuls need goes first
        load_w(wq_sb, wq_d, 0, DC // 2)
        load_w(wq_sb, wq_d, DC // 2, DC)
        load_xh(0, 0)
        load_w(wk_sb, wk_d, 0, DC // 2)
        load_w(wk_sb, wk_d, DC // 2, DC)
        load_w(wv_sb, wv_d, 0, DC // 2)
        load_w(wv_sb, wv_d, DC // 2, DC)
        nc.sync.dma_start(cs1_sb[:], cs1_d[:])
        nc.sync.dma_start(cs2_sb[:], cs2_d[:])
        nc.sync.dma_start(ones_sb[:], ones_d[:])
        load_xh(0, 1)
        # wo only needed at first outproj (~90us in)
        nc.sync.dma_start(wo_sb[:], wo_d[:])

        # deferred output projection: [g0, ib, next-tile-cursor] or None
        pend = [None]

        def emit_yp(k=None, alt=False):
            # one outproj psum tile of the pending i-block (k overrides cursor)
            if pend[0] is None:
                return
            g0, ib, cur = pend[0]
            if k is None:
                k = cur
                pend[0] = None if cur + 1 == NYT else [g0, ib, cur + 1]
            tl, db = k // (D // IBLK), k % (D // IBLK)
            tt = ib * (IBLK // P) + tl
            yp = yps.tile([P, IBLK], f32, tag="y")
            for h in range(HPC):
                nc.tensor.matmul(
                    yp[:],
                    oT_sb[:, h * T + tt * P:h * T + (tt + 1) * P],
                    wo_sb[:, h * D + db * IBLK:h * D + (db + 1) * IBLK],
                    start=(h == 0), stop=(h == HPC - 1))
            yt = ypool.tile([P, IBLK], f32, tag="yt")
            # steady state: scalar takes 2/8 (exp-limited); final drain:
            # alternate so the copies pipeline two wide
            on_scalar = (k % 2 == 0) if alt else (k % 8 < 2)
            if on_scalar:
                nc.scalar.copy(yt[:], yp[:])
            else:
                nc.vector.tensor_scalar_mul(yt[:], yp[:], 1.0)
            nc.sync.dma_start(
                y_d[g0 + tt * P:g0 + (tt + 1) * P,
                    db * IBLK:(db + 1) * IBLK],
                yt[:])

        for b in range(B):
            g0 = b * T

            with tc.tile_pool(name=f"qk_ps{b}", bufs=4, space="PSUM") as qkps, \
                 tc.tile_pool(name=f"v_ps{b}", bufs=1, space="PSUM") as vps:
                for blk in range(NBLK):
                    t0 = blk * TBLK
                    xt = xtiles[(b, blk // 2)]
                    xc0 = (blk % 2) * TBLK  # column offset inside the hb tile

                    for h in range(HPC):
                        for w_sb, dst in ((wq_sb, qT_sb), (wk_sb, kT_sb)):
                            pps = qkps.tile([P, TBLK], f32, tag="qk")
                            for dc in range(DC):
                                nc.tensor.matmul(
                                    pps[:],
                                    w_sb[:, dc * CPC + h * HD:dc * CPC + (h + 1) * HD],
                                    xt[:, dc * HB + xc0:dc * HB + xc0 + TBLK],
                                    start=(dc == 0), stop=(dc == DC - 1))
                            m1 = rpool.tile([P, TBLK], f32, tag="m1")
                            m3 = rpool.tile([P, TBLK], f32, tag="m3")
                            c1 = cs1_sb[:, t0:t0 + TBLK]
                            c2 = cs2_sb[:, t0:t0 + TBLK]
                            nc.vector.tensor_tensor(m1[:], pps[:], c1, mult)
                            nc.vector.tensor_tensor(m3[:], pps[:], c2, mult)
                            sw = rpool.tile([P, TBLK], f32, tag="sw")
                            nc.sync.dma_start(sw[0:64, :], m1[64:128, :])
                            nc.sync.dma_start(sw[64:128, :], m3[0:64, :])
                            o = dst[:, h * T + t0:h * T + t0 + TBLK]
                            nc.vector.tensor_tensor(
                                o[0:64, :], m1[0:64, :], sw[0:64, :], sub)
                            nc.vector.tensor_tensor(
                                o[64:128, :], m3[64:128, :], sw[64:128, :], add)
                            emit_yp()  # prev batch's last outproj block

                    vp = vps.tile([P, 4 * CPC], f32, tag="v")
                    for tl in range(TBLK // P):
                        for dc in range(DC):
                            nc.tensor.matmul(
                                vp[:, tl * CPC:(tl + 1) * CPC],
                                xt[:, dc * HB + xc0 + tl * P:dc * HB + xc0 + (tl + 1) * P],
                                wv_sb[:, dc * CPC:(dc + 1) * CPC],
                                start=(dc == 0), stop=(dc == DC - 1))
                    nc.scalar.copy(
                        v_sb[:, blk * 4 * CPC:(blk + 1) * 4 * CPC], vp[:])

            # prefetch next batch's x during this batch's attention
            if b + 1 < B:
                load_xh(b + 1, 0)
                load_xh(b + 1, 1)

            with tc.tile_pool(name=f"s_ps{b}", bufs=2, space="PSUM") as sps, \
                 tc.tile_pool(name=f"o_ps{b}", bufs=1, space="PSUM") as ops, \
                 tc.tile_pool(name=f"d_ps{b}", bufs=1, space="PSUM") as dps:
                for ib in range(NIB):
                    i0 = ib * IBLK
                    for h in range(HPC):
                        q_sl = qT_sb[:, h * T + i0:h * T + i0 + IBLK]
                        op = ops.tile([P, IBLK], f32, tag="o")
                        accA = accpool.tile([P, 2 * IBLK], bf16, tag="accA")
                        for pr in range(NPR):
                            j0, j1 = 2 * pr, 2 * pr + 1
                            sp = sps.tile([P, 2 * IBLK], f32, tag="s")
                            nc.tensor.matmul(
                                sp[:, 0:IBLK],
                                kT_sb[:, h * T + j0 * P:h * T + (j0 + 1) * P],
                                q_sl, start=True, stop=True)
                            nc.tensor.matmul(
                                sp[:, IBLK:2 * IBLK],
                                kT_sb[:, h * T + j1 * P:h * T + (j1 + 1) * P],
                                q_sl, start=True, stop=True)
                            e = epool.tile([P, 2 * IBLK], bf16, tag="e")
                            nc.scalar.activation(e[:], sp[:], Exp, scale=SCALE)
                            nc.tensor.matmul(
                                op[:],
                                v_sb[:, j0 * CPC + h * HD:j0 * CPC + (h + 1) * HD],
                                e[:, 0:IBLK], start=(pr == 0), stop=False)
                            nc.tensor.matmul(
                                op[:],
                                v_sb[:, j1 * CPC + h * HD:j1 * CPC + (h + 1) * HD],
                                e[:, IBLK:2 * IBLK],
                                start=False, stop=(pr == NPR - 1))
                            # denominator partial sum on DVE (bf16 2x mode;
                            # gpsimd measures ~7x slower than its cost model)
                            if pr == 0:
                                nc.vector.tensor_scalar_mul(accA[:], e[:], 1.0)
                            else:
                                nc.vector.tensor_tensor(accA[:], accA[:], e[:], add)
                            emit_yp()  # interleave prev i-block's outproj
                        dn = dps.tile([P, IBLK], f32, tag="d")
                        nc.tensor.matmul(dn[:], ones_sb[:], accA[:, 0:IBLK],
                                         start=True, stop=False)
                        nc.tensor.matmul(dn[:], ones_sb[:], accA[:, IBLK:2 * IBLK],
                                         start=False, stop=True)
                        rcp = rcpool.tile([P, IBLK], f32, tag="rc")
                        nc.vector.reciprocal_approx_fast(out=rcp[:], in_=dn[:])
                        nc.vector.tensor_tensor(
                            oT_sb[:, h * T + i0:h * T + i0 + IBLK],
                            op[:], rcp[:], mult)
                    while pend[0] is not None:  # drain any leftovers
                        emit_yp()
                    pend[0] = [g0, ib, 0]

        for k in range(NYT):  # final i-block's projection (tail)
            emit_yp(k, alt=True)


_EVEN_ODD = np.concatenate([np.arange(0, HD, 2), np.arange(1, HD, 2)])
_BF16 = ml_dtypes.bfloat16


def _pack_w(w):
    # [D, CPC] -> SBUF image [P, DC*CPC] (row p holds dc-major chunks)
    return np.ascontiguousarray(
        w.reshape(DC, P, CPC).transpose(1, 0, 2).reshape(P, DC * CPC)
        .astype(_BF16))


def _prep_inputs(x, rope_cos, rope_sin, Wq, Wk, Wv, Wo):
    x = np.asarray(x, dtype=np.float32)
    xT = x.reshape(BT, D).T  # [D, BT]
    # pack into per-half-batch SBUF images: row hb*P+p, col dc*HB+t
    xpk = np.ascontiguousarray(
        xT.reshape(DC, P, BT // HB, HB).transpose(2, 1, 0, 3)
        .reshape((BT // HB) * P, DC * HB).astype(_BF16))
    cosT = np.asarray(rope_cos, dtype=np.float32).T
    sinT = np.asarray(rope_sin, dtype=np.float32).T
    cs1 = np.ascontiguousarray(
        np.concatenate([cosT, sinT], axis=0), dtype=np.float32)
    cs2 = np.ascontiguousarray(
        np.concatenate([sinT, cosT], axis=0), dtype=np.float32)
    ones = np.ones((P, P), dtype=_BF16)
    Wq = np.asarray(Wq, dtype=np.float32)
    Wk = np.asarray(Wk, dtype=np.float32)
    Wv = np.asarray(Wv, dtype=np.float32)
    Wo = np.asarray(Wo, dtype=np.float32)

    in_maps = []
    for c in range(NCORES):
        cols = slice(c * CPC, (c + 1) * CPC)
        wq_c = Wq[:, cols].reshape(D, HPC, HD)[:, :, _EVEN_ODD].reshape(D, CPC)
        wk_c = Wk[:, cols].reshape(D, HPC, HD)[:, :, _EVEN_ODD].reshape(D, CPC)
        wo_c = Wo[cols, :]  # [CPC, D] -> [P, HPC*D]
        in_maps.append({
            "xT": xpk,
            "wq": _pack_w(wq_c),
            "wk": _pack_w(wk_c),
            "wv": _pack_w(Wv[:, cols]),
            "wo": np.ascontiguousarray(
                wo_c.reshape(HPC, P, D).transpose(1, 0, 2)
                .reshape(P, HPC * D).astype(_BF16)),
            "cs1": cs1,
            "cs2": cs2,
            "ones": ones,
        })
    return in_maps


def kernel(x, rope_cos, rope_sin, Wq, Wk, Wv, Wo, _trace=False):
    global last_results
    if "nc" not in _compiled:
        _compiled["nc"] = _build()
    nc = _compiled["nc"]
    in_maps = _prep_inputs(x, rope_cos, rope_sin, Wq, Wk, Wv, Wo)
    res = run_bass_kernel_spmd(
        nc, in_maps, core_ids=list(range(NCORES)), trace=_trace)
    last_results = res
    y = np.sum(np.stack([res.results[c]["y"] for c in range(NCORES)]),
               axis=0, dtype=np.float64)
    return y.reshape(B, T, D).astype(np.float32)


# revision 17
# speedup vs baseline: 1.0115x; 1.0115x over previous
"""Tensor-parallel causal-self-attention (full-attention) Bass kernel for TRN2.

Sharding: 16 heads over 8 cores (2 heads/core). Each core computes its heads'
QKV projections, rope, full attention, and its partial output projection
(rows of Wo for its heads); the host sums the 8 partial outputs (the
all-reduce of the tensor-parallel pattern, done at gather time).

Per-core layouts (everything "transposed", tokens on the free axis). All
matmul operands are bf16 (same PE rate as f32r, half the DMA/SBUF), psum f32:
  xT      [D=2048, B*T=4096]   x transposed (host-prepped, bf16), replicated
  wq/wk   [2048, 256]          head-column shard; within each head the 128
                               columns are permuted evens-then-odds so rope
                               pairs become contiguous partition halves
  wv      [2048, 256]          natural column shard
  wo      [256, 2048]          natural row shard
  cs1     [128, 2048]          [cos.T ; sin.T] stacked (64+64 partitions), f32
  cs2     [128, 2048]          [sin.T ; cos.T]
  ones    [128, 128]           all-ones (softmax denominator broadcast matmul)

Initial DMAs are issued fine-grained in priority order (wq, x half-batch 0,
wk, wv, cs, wo) so the first matmul starts early; x is loaded in half-batch
tiles (1024 tokens -> 2KB DMA lines) chunked 4-ways for subtile overlap.

Pipeline per batch b in {0,1}:
  A) qT/kT = W.T @ xT (per head, 512-token psum blocks, N=512 matmuls);
     rope via partition-half realign DMAs; v = xT.T @ wv (natural
     [token,256] tiles).  The previous batch's last output-projection block
     is interleaved one tile per rope group.
  B) per (head, i-block of 512): score PAIRS into a 2-bank psum tile
     ([128,1024]); ONE exp ACT per pair (psum->sbuf bf16); oT += v.T @ e.
     Softmax denominator: e-pairs are accumulated into two partial sums
     (DVE takes even pairs, gpsimd odd pairs, both bf16 SBUF) and a single
     ones-matmul pair broadcasts the partition-sum — replaces 16 full
     denominator matmuls per (h,ib) with 2 (saves ~50us of PE streaming).
     oT_norm = op * recip_approx(denom).
  C) y[t,d] = sum_h oT_h.T @ wo_h. The i-block's projection is deferred one
     i-block and its 16 psum tiles are emitted round-robin between score/PV
     pairs, so the PE (not the exp ACT) paces the attention inner loop.
     psum->sbuf staging copies split scalar/vector; DMA out (partial sums).
"""

import sys

sys.path.insert(0, "/opt/trn_rl_repo")

import numpy as np
import ml_dtypes

import concourse.bass as bass
import concourse.mybir as mybir
import concourse.tile as tile
from concourse import bacc
from concourse.bass_utils import run_bass_kernel_spmd

B, T, D = 2, 2048, 2048
NH, HD = 16, 128
NCORES = 8
HPC = NH // NCORES          # heads per core = 2
CPC = HPC * HD              # proj columns per core = 256
BT = B * T                  # 4096 tokens
P = 128
TBLK = 512                  # phase-A token block
NBLK = T // TBLK            # 4 blocks per batch
HB = 2 * TBLK               # x half-batch tile token width (1024)
DC = D // P                 # 16 contraction chunks
IBLK = 512                  # phase-B query block
NIB = T // IBLK             # 4 i-blocks per batch
NJT = T // P                # 16 key tiles per batch
NPR = NJT // 2              # 8 key-tile pairs
NYT = (IBLK // P) * (D // IBLK)  # outproj psum tiles per i-block = 16
SCALE = 1.0 / float(np.sqrt(HD))

f32 = mybir.dt.float32
bf16 = mybir.dt.bfloat16

_compiled = {}

# exposed for test.py
last_results = None


def _build():
    nc = bacc.Bacc("TRN2", target_bir_lowering=False, debug=False)

    # all weights and x arrive host-prepacked as their exact SBUF images so
    # every DMA is contiguous per partition (8-32KB lines, full bandwidth)
    xT_d = nc.dram_tensor("xT", [(BT // HB) * P, DC * HB], bf16,
                          kind="ExternalInput").ap()
    wq_d = nc.dram_tensor("wq", [P, DC * CPC], bf16, kind="ExternalInput").ap()
    wk_d = nc.dram_tensor("wk", [P, DC * CPC], bf16, kind="ExternalInput").ap()
    wv_d = nc.dram_tensor("wv", [P, DC * CPC], bf16, kind="ExternalInput").ap()
    wo_d = nc.dram_tensor("wo", [P, HPC * D], bf16, kind="ExternalInput").ap()
    cs1_d = nc.dram_tensor("cs1", [P, T], f32, kind="ExternalInput").ap()
    cs2_d = nc.dram_tensor("cs2", [P, T], f32, kind="ExternalInput").ap()
    ones_d = nc.dram_tensor("ones", [P, P], bf16, kind="ExternalInput").ap()
    y_d = nc.dram_tensor("y", [BT, D], bf16, kind="ExternalOutput").ap()

    with tile.TileContext(nc) as tc:
        _emit(nc, tc, xT_d, wq_d, wk_d, wv_d, wo_d, cs1_d, cs2_d, ones_d, y_d)
    nc.compile()
    return nc


def _emit(nc, tc, xT_d, wq_d, wk_d, wv_d, wo_d, cs1_d, cs2_d, ones_d, y_d):
    from contextlib import ExitStack

    Exp = mybir.ActivationFunctionType.Exp
    mult = mybir.AluOpType.mult
    add = mybir.AluOpType.add
    sub = mybir.AluOpType.subtract

    with ExitStack() as ctx:
        const = ctx.enter_context(tc.tile_pool(name="const", bufs=1))
        state = ctx.enter_context(tc.tile_pool(name="state", bufs=1))
        xpool = ctx.enter_context(tc.tile_pool(name="xa", bufs=2))
        rpool = ctx.enter_context(tc.tile_pool(name="ra", bufs=4))
        epool = ctx.enter_context(tc.tile_pool(name="e", bufs=3))
        accpool = ctx.enter_context(tc.tile_pool(name="ac", bufs=2))
        rcpool = ctx.enter_context(tc.tile_pool(name="rc", bufs=2))
        ypool = ctx.enter_context(tc.tile_pool(name="yb", bufs=4))
        yps = ctx.enter_context(tc.tile_pool(name="y_ps", bufs=2, space="PSUM"))

        wq_sb = const.tile([P, DC * CPC], bf16, tag="wq")
        wk_sb = const.tile([P, DC * CPC], bf16, tag="wk")
        wv_sb = const.tile([P, DC * CPC], bf16, tag="wv")
        wo_sb = const.tile([P, HPC * D], bf16, tag="wo")
        cs1_sb = const.tile([P, T], f32, tag="cs1")
        cs2_sb = const.tile([P, T], f32, tag="cs2")
        ones_sb = const.tile([P, P], bf16, tag="ones")

        qT_sb = state.tile([P, HPC * T], bf16, tag="qT")
        kT_sb = state.tile([P, HPC * T], bf16, tag="kT")
        v_sb = state.tile([P, NJT * CPC], bf16, tag="v")
        oT_sb = state.tile([P, HPC * T], bf16, tag="oT")

        def load_w(sb, dr, c0, c1, eng=None):
            # dc-chunk [c0, c1) of a prepacked weight image (contiguous)
            (eng or nc.sync).dma_start(sb[:, c0 * CPC:c1 * CPC],
                                       dr[:, c0 * CPC:c1 * CPC])

        xtiles = {}

        def load_xh(b, half, eng=None):
            # half-batch tile (1024 tokens), prepacked: contiguous 8KB chunks
            hbi = b * 2 + half
            xt = xpool.tile([P, DC * HB], bf16, tag="x")
            for c0, c1 in ((0, 4), (4, 8), (8, 12), (12, 16)):
                (eng or nc.sync).dma_start(
                    xt[:, c0 * HB:c1 * HB],
                    xT_d[hbi * P:(hbi + 1) * P, c0 * HB:c1 * HB])
            xtiles[(b, half)] = xt

        # priority-ordered startup, issue split between the two HWDGE
        # sequencers (sync + scalar) so the ~0.7us-per-DMA issue cost
        # doesn't serialize the critical prefix
        load_w(wq_sb, wq_d, 0, DC // 2)            # sync: first QK chain
        load_xh(0, 0)                              # sync: first x half-batch
        load_w(wq_sb, wq_d, DC // 2, DC, nc.scalar)
        load_w(wk_sb, wk_d, 0, DC // 2, nc.scalar)
        load_w(wk_sb, wk_d, DC // 2, DC, nc.scalar)
        load_w(wv_sb, wv_d, 0, DC // 2, nc.scalar)
        load_w(wv_sb, wv_d, DC // 2, DC, nc.scalar)
        nc.sync.dma_start(cs1_sb[:], cs1_d[:])
        nc.sync.dma_start(cs2_sb[:], cs2_d[:])
        nc.sync.dma_start(ones_sb[:], ones_d[:])
        load_xh(0, 1, nc.scalar)
        # wo only needed at first outproj (~90us in)
        nc.scalar.dma_start(wo_sb[:], wo_d[:])

        # deferred output projection: [g0, ib, next-tile-cursor] or None
        pend = [None]

        def emit_yp(k=None, alt=False):
            # one outproj psum tile of the pending i-block (k overrides cursor)
            if pend[0] is None:
                return
            g0, ib, cur = pend[0]
            if k is None:
                k = cur
                pend[0] = None if cur + 1 == NYT else [g0, ib, cur + 1]
            tl, db = k // (D // IBLK), k % (D // IBLK)
            tt = ib * (IBLK // P) + tl
            yp = yps.tile([P, IBLK], f32, tag="y")
            for h in range(HPC):
                nc.tensor.matmul(
                    yp[:],
                    oT_sb[:, h * T + tt * P:h * T + (tt + 1) * P],
                    wo_sb[:, h * D + db * IBLK:h * D + (db + 1) * IBLK],
                    start=(h == 0), stop=(h == HPC - 1))
            yt = ypool.tile([P, IBLK], bf16, tag="yt")
            # steady state: scalar takes 2/8 (exp-limited); final drain:
            # alternate so the copies pipeline two wide
            on_scalar = (k % 2 == 0) if alt else (k % 8 < 2)
            if on_scalar:
                nc.scalar.copy(yt[:], yp[:])
            else:
                nc.vector.tensor_scalar_mul(yt[:], yp[:], 1.0)
            nc.sync.dma_start(
                y_d[g0 + tt * P:g0 + (tt + 1) * P,
                    db * IBLK:(db + 1) * IBLK],
                yt[:])

        for b in range(B):
            g0 = b * T

            with tc.tile_pool(name=f"qk_ps{b}", bufs=4, space="PSUM") as qkps, \
                 tc.tile_pool(name=f"v_ps{b}", bufs=1, space="PSUM") as vps:
                for blk in range(NBLK):
                    t0 = blk * TBLK
                    xt = xtiles[(b, blk // 2)]
                    xc0 = (blk % 2) * TBLK  # column offset inside the hb tile

                    for h in range(HPC):
                        for w_sb, dst in ((wq_sb, qT_sb), (wk_sb, kT_sb)):
                            pps = qkps.tile([P, TBLK], f32, tag="qk")
                            for dc in range(DC):
                                nc.tensor.matmul(
                                    pps[:],
                                    w_sb[:, dc * CPC + h * HD:dc * CPC + (h + 1) * HD],
                                    xt[:, dc * HB + xc0:dc * HB + xc0 + TBLK],
                                    start=(dc == 0), stop=(dc == DC - 1))
                            m1 = rpool.tile([P, TBLK], f32, tag="m1")
                            m3 = rpool.tile([P, TBLK], f32, tag="m3")
                            c1 = cs1_sb[:, t0:t0 + TBLK]
                            c2 = cs2_sb[:, t0:t0 + TBLK]
                            nc.vector.tensor_tensor(m1[:], pps[:], c1, mult)
                            nc.vector.tensor_tensor(m3[:], pps[:], c2, mult)
                            sw = rpool.tile([P, TBLK], f32, tag="sw")
                            nc.sync.dma_start(sw[0:64, :], m1[64:128, :])
                            nc.sync.dma_start(sw[64:128, :], m3[0:64, :])
                            o = dst[:, h * T + t0:h * T + t0 + TBLK]
                            nc.vector.tensor_tensor(
                                o[0:64, :], m1[0:64, :], sw[0:64, :], sub)
                            nc.vector.tensor_tensor(
                                o[64:128, :], m3[64:128, :], sw[64:128, :], add)
                            emit_yp()  # prev batch's last outproj block

                    vp = vps.tile([P, 4 * CPC], f32, tag="v")
                    for tl in range(TBLK // P):
                        for dc in range(DC):
                            nc.tensor.matmul(
                                vp[:, tl * CPC:(tl + 1) * CPC],
                                xt[:, dc * HB + xc0 + tl * P:dc * HB + xc0 + (tl + 1) * P],
                                wv_sb[:, dc * CPC:(dc + 1) * CPC],
                                start=(dc == 0), stop=(dc == DC - 1))
                    nc.scalar.copy(
                        v_sb[:, blk * 4 * CPC:(blk + 1) * 4 * CPC], vp[:])

            # prefetch next batch's x during this batch's attention
            if b + 1 < B:
                load_xh(b + 1, 0)
                load_xh(b + 1, 1)

            with tc.tile_pool(name=f"s_ps{b}", bufs=2, space="PSUM") as sps, \
                 tc.tile_pool(name=f"o_ps{b}", bufs=1, space="PSUM") as ops, \
                 tc.tile_pool(name=f"d_ps{b}", bufs=1, space="PSUM") as dps:
                for ib in range(NIB):
                    i0 = ib * IBLK
                    for h in range(HPC):
                        q_sl = qT_sb[:, h * T + i0:h * T + i0 + IBLK]
                        op = ops.tile([P, IBLK], f32, tag="o")
                        accA = accpool.tile([P, 2 * IBLK], bf16, tag="accA")
                        for pr in range(NPR):
                            j0, j1 = 2 * pr, 2 * pr + 1
                            sp = sps.tile([P, 2 * IBLK], f32, tag="s")
                            nc.tensor.matmul(
                                sp[:, 0:IBLK],
                                kT_sb[:, h * T + j0 * P:h * T + (j0 + 1) * P],
                                q_sl, start=True, stop=True)
                            nc.tensor.matmul(
                                sp[:, IBLK:2 * IBLK],
                                kT_sb[:, h * T + j1 * P:h * T + (j1 + 1) * P],
                                q_sl, start=True, stop=True)
                            e = epool.tile([P, 2 * IBLK], bf16, tag="e")
                            nc.scalar.activation(e[:], sp[:], Exp, scale=SCALE)
                            nc.tensor.matmul(
                                op[:],
                                v_sb[:, j0 * CPC + h * HD:j0 * CPC + (h + 1) * HD],
                                e[:, 0:IBLK], start=(pr == 0), stop=False)
                            nc.tensor.matmul(
                                op[:],
                                v_sb[:, j1 * CPC + h * HD:j1 * CPC + (h + 1) * HD],
                                e[:, IBLK:2 * IBLK],
                                start=False, stop=(pr == NPR - 1))
                            # denominator partial sum on DVE (bf16 2x mode;
                            # gpsimd measures ~7x slower than its cost model)
                            if pr == 0:
                                nc.vector.tensor_scalar_mul(accA[:], e[:], 1.0)
                            else:
                                nc.vector.tensor_tensor(accA[:], accA[:], e[:], add)
                            emit_yp()  # interleave prev i-block's outproj
                        dn = dps.tile([P, IBLK], f32, tag="d")
                        nc.tensor.matmul(dn[:], ones_sb[:], accA[:, 0:IBLK],
                                         start=True, stop=False)
                        nc.tensor.matmul(dn[:], ones_sb[:], accA[:, IBLK:2 * IBLK],
                                         start=False, stop=True)
                        rcp = rcpool.tile([P, IBLK], f32, tag="rc")
                        nc.vector.reciprocal_approx_fast(out=rcp[:], in_=dn[:])
                        nc.vector.tensor_tensor(
                            oT_sb[:, h * T + i0:h * T + i0 + IBLK],
                            op[:], rcp[:], mult)
                    while pend[0] is not None:  # drain any leftovers
                        emit_yp()
                    pend[0] = [g0, ib, 0]

        for k in range(NYT):  # final i-block's projection (tail)
            emit_yp(k, alt=True)


_EVEN_ODD = np.concatenate([np.arange(0, HD, 2), np.arange(1, HD, 2)])
_BF16 = ml_dtypes.bfloat16


def _pack_w(w):
    # [D, CPC] -> SBUF image [P, DC*CPC] (row p holds dc-major chunks)
    return np.ascontiguousarray(
        w.reshape(DC, P, CPC).transpose(1, 0, 2).reshape(P, DC * CPC)
        .astype(_BF16))


def _prep_inputs(x, rope_cos, rope_sin, Wq, Wk, Wv, Wo):
    x = np.asarray(x, dtype=np.float32)
    xT = x.reshape(BT, D).T  # [D, BT]
    # pack into per-half-batch SBUF images: row hb*P+p, col dc*HB+t
    xpk = np.ascontiguousarray(
        xT.reshape(DC, P, BT // HB, HB).transpose(2, 1, 0, 3)
        .reshape((BT // HB) * P, DC * HB).astype(_BF16))
    cosT = np.asarray(rope_cos, dtype=np.float32).T
    sinT = np.asarray(rope_sin, dtype=np.float32).T
    cs1 = np.ascontiguousarray(
        np.concatenate([cosT, sinT], axis=0), dtype=np.float32)
    cs2 = np.ascontiguousarray(
        np.concatenate([sinT, cosT], axis=0), dtype=np.float32)
    ones = np.ones((P, P), dtype=_BF16)
    Wq = np.asarray(Wq, dtype=np.float32)
    Wk = np.asarray(Wk, dtype=np.float32)
    Wv = np.asarray(Wv, dtype=np.float32)
    Wo = np.asarray(Wo, dtype=np.float32)

    in_maps = []
    for c in range(NCORES):
        cols = slice(c * CPC, (c + 1) * CPC)
        wq_c = Wq[:, cols].reshape(D, HPC, HD)[:, :, _EVEN_ODD].reshape(D, CPC)
        wk_c = Wk[:, cols].reshape(D, HPC, HD)[:, :, _EVEN_ODD].reshape(D, CPC)
        wo_c = Wo[cols, :]  # [CPC, D] -> [P, HPC*D]
        in_maps.append({
            "xT": xpk,
            "wq": _pack_w(wq_c),
            "wk": _pack_w(wk_c),
            "wv": _pack_w(Wv[:, cols]),
            "wo": np.ascontiguousarray(
                wo_c.reshape(HPC, P, D).transpose(1, 0, 2)
                .reshape(P, HPC * D).astype(_BF16)),
            "cs1": cs1,
            "cs2": cs2,
            "ones": ones,
        })
    return in_maps


def kernel(x, rope_cos, rope_sin, Wq, Wk, Wv, Wo, _trace=False):
    global last_results
    if "nc" not in _compiled:
        _compiled["nc"] = _build()
    nc = _compiled["nc"]
    in_maps = _prep_inputs(x, rope_cos, rope_sin, Wq, Wk, Wv, Wo)
    res = run_bass_kernel_spmd(
        nc, in_maps, core_ids=list(range(NCORES)), trace=_trace)
    last_results = res
    y = np.sum(np.stack([res.results[c]["y"].astype(np.float64)
                         for c in range(NCORES)]), axis=0)
    return y.reshape(B, T, D).astype(np.float32)


# revision 20
# speedup vs baseline: 1.0139x; 1.0023x over previous
"""Tensor-parallel causal-self-attention (full-attention) Bass kernel for TRN2.

Sharding: 16 heads over 8 cores (2 heads/core). Each core computes its heads'
QKV projections, rope, full attention, and its partial output projection
(rows of Wo for its heads); the host sums the 8 partial outputs (the
all-reduce of the tensor-parallel pattern, done at gather time).

Per-core layouts (everything "transposed", tokens on the free axis). All
matmul operands are bf16 (same PE rate as f32r, half the DMA/SBUF), psum f32:
  xT      [D=2048, B*T=4096]   x transposed (host-prepped, bf16), replicated
  wq/wk   [2048, 256]          head-column shard; within each head the 128
                               columns are permuted evens-then-odds so rope
                               pairs become contiguous partition halves
  wv      [2048, 256]          natural column shard
  wo      [256, 2048]          natural row shard
  cs1     [128, 2048]          [cos.T ; sin.T] stacked (64+64 partitions), f32
  cs2     [128, 2048]          [sin.T ; cos.T]
  ones    [128, 128]           all-ones (softmax denominator broadcast matmul)

Initial DMAs are issued fine-grained in priority order (wq, x half-batch 0,
wk, wv, cs, wo) so the first matmul starts early; x is loaded in half-batch
tiles (1024 tokens -> 2KB DMA lines) chunked 4-ways for subtile overlap.

Pipeline per batch b in {0,1}:
  A) qT/kT = W.T @ xT (per head, 512-token psum blocks, N=512 matmuls);
     rope via partition-half realign DMAs; v = xT.T @ wv (natural
     [token,256] tiles).  The previous batch's last output-projection block
     is interleaved one tile per rope group.
  B) per (head, i-block of 512): score PAIRS into a 2-bank psum tile
     ([128,1024]); ONE exp ACT per pair (psum->sbuf bf16); oT += v.T @ e.
     Softmax denominator: e-pairs are accumulated into two partial sums
     (DVE takes even pairs, gpsimd odd pairs, both bf16 SBUF) and a single
     ones-matmul pair broadcasts the partition-sum — replaces 16 full
     denominator matmuls per (h,ib) with 2 (saves ~50us of PE streaming).
     oT_norm = op * recip_approx(denom).
  C) y[t,d] = sum_h oT_h.T @ wo_h. The i-block's projection is deferred one
     i-block and its 16 psum tiles are emitted round-robin between score/PV
     pairs, so the PE (not the exp ACT) paces the attention inner loop.
     psum->sbuf staging copies split scalar/vector; DMA out (partial sums).
"""

import sys

sys.path.insert(0, "/opt/trn_rl_repo")

import numpy as np
import ml_dtypes

import concourse.bass as bass
import concourse.mybir as mybir
import concourse.tile as tile
from concourse import bacc
from concourse.bass_utils import run_bass_kernel_spmd

B, T, D = 2, 2048, 2048
NH, HD = 16, 128
NCORES = 8
HPC = NH // NCORES          # heads per core = 2
CPC = HPC * HD              # proj columns per core = 256
BT = B * T                  # 4096 tokens
P = 128
TBLK = 512                  # phase-A token block
NBLK = T // TBLK            # 4 blocks per batch
HB = 2 * TBLK               # x half-batch tile token width (1024)
DC = D // P                 # 16 contraction chunks
IBLK = 512                  # phase-B query block
NIB = T // IBLK             # 4 i-blocks per batch
NJT = T // P                # 16 key tiles per batch
NPR = NJT // 2              # 8 key-tile pairs
NYT = (IBLK // P) * (D // IBLK)  # outproj psum tiles per i-block = 16
SCALE = 1.0 / float(np.sqrt(HD))

f32 = mybir.dt.float32
bf16 = mybir.dt.bfloat16

_compiled = {}

# exposed for test.py
last_results = None


def _build():
    nc = bacc.Bacc("TRN2", target_bir_lowering=False, debug=False)

    # all weights and x arrive host-prepacked as their exact SBUF images so
    # every DMA is contiguous per partition (8-32KB lines, full bandwidth)
    xT_d = nc.dram_tensor("xT", [(BT // HB) * P, DC * HB], bf16,
                          kind="ExternalInput").ap()
    wq_d = nc.dram_tensor("wq", [P, DC * CPC], bf16, kind="ExternalInput").ap()
    wk_d = nc.dram_tensor("wk", [P, DC * CPC], bf16, kind="ExternalInput").ap()
    wv_d = nc.dram_tensor("wv", [P, DC * CPC], bf16, kind="ExternalInput").ap()
    wo_d = nc.dram_tensor("wo", [P, HPC * D], bf16, kind="ExternalInput").ap()
    cs1_d = nc.dram_tensor("cs1", [P, T], f32, kind="ExternalInput").ap()
    cs2_d = nc.dram_tensor("cs2", [P, T], f32, kind="ExternalInput").ap()
    ones_d = nc.dram_tensor("ones", [P, P], bf16, kind="ExternalInput").ap()
    y_d = nc.dram_tensor("y", [BT, D], bf16, kind="ExternalOutput").ap()

    with tile.TileContext(nc) as tc:
        _emit(nc, tc, xT_d, wq_d, wk_d, wv_d, wo_d, cs1_d, cs2_d, ones_d, y_d)
    nc.compile()
    return nc


def _emit(nc, tc, xT_d, wq_d, wk_d, wv_d, wo_d, cs1_d, cs2_d, ones_d, y_d):
    from contextlib import ExitStack

    Exp = mybir.ActivationFunctionType.Exp
    mult = mybir.AluOpType.mult
    add = mybir.AluOpType.add
    sub = mybir.AluOpType.subtract

    with ExitStack() as ctx:
        const = ctx.enter_context(tc.tile_pool(name="const", bufs=1))
        state = ctx.enter_context(tc.tile_pool(name="state", bufs=1))
        xpool = ctx.enter_context(tc.tile_pool(name="xa", bufs=2))
        rpool = ctx.enter_context(tc.tile_pool(name="ra", bufs=4))
        epool = ctx.enter_context(tc.tile_pool(name="e", bufs=3))
        accpool = ctx.enter_context(tc.tile_pool(name="ac", bufs=2))
        rcpool = ctx.enter_context(tc.tile_pool(name="rc", bufs=2))
        ypool = ctx.enter_context(tc.tile_pool(name="yb", bufs=4))
        yps = ctx.enter_context(tc.tile_pool(name="y_ps", bufs=2, space="PSUM"))

        wq_sb = const.tile([P, DC * CPC], bf16, tag="wq")
        wk_sb = const.tile([P, DC * CPC], bf16, tag="wk")
        wv_sb = const.tile([P, DC * CPC], bf16, tag="wv")
        wo_sb = const.tile([P, HPC * D], bf16, tag="wo")
        cs1_sb = const.tile([P, T], f32, tag="cs1")
        cs2_sb = const.tile([P, T], f32, tag="cs2")
        ones_sb = const.tile([P, P], bf16, tag="ones")

        qT_sb = state.tile([P, HPC * T], bf16, tag="qT")
        kT_sb = state.tile([P, HPC * T], bf16, tag="kT")
        v_sb = state.tile([P, NJT * CPC], bf16, tag="v")
        oT_sb = state.tile([P, HPC * T], bf16, tag="oT")

        def load_w(sb, dr, c0, c1, eng=None):
            # dc-chunk [c0, c1) of a prepacked weight image (contiguous)
            (eng or nc.sync).dma_start(sb[:, c0 * CPC:c1 * CPC],
                                       dr[:, c0 * CPC:c1 * CPC])

        xtiles = {}

        def load_xh(b, half, eng=None):
            # half-batch tile (1024 tokens), prepacked: contiguous 8KB chunks
            hbi = b * 2 + half
            xt = xpool.tile([P, DC * HB], bf16, tag="x")
            for c0, c1 in ((0, 4), (4, 8), (8, 12), (12, 16)):
                (eng or nc.sync).dma_start(
                    xt[:, c0 * HB:c1 * HB],
                    xT_d[hbi * P:(hbi + 1) * P, c0 * HB:c1 * HB])
            xtiles[(b, half)] = xt

        # priority-ordered startup, issue split between the two HWDGE
        # sequencers (sync + scalar) so the ~0.7us-per-DMA issue cost
        # doesn't serialize the critical prefix
        load_xh(0, 0)                              # sync: first x half-batch
        load_w(wq_sb, wq_d, 0, DC // 2, nc.scalar)  # scalar: first QK chain
        load_w(wq_sb, wq_d, DC // 2, DC, nc.scalar)
        load_w(wk_sb, wk_d, 0, DC // 2, nc.scalar)
        load_w(wk_sb, wk_d, DC // 2, DC, nc.scalar)
        load_w(wv_sb, wv_d, 0, DC // 2, nc.scalar)
        load_w(wv_sb, wv_d, DC // 2, DC, nc.scalar)
        nc.sync.dma_start(cs1_sb[:], cs1_d[:])
        nc.sync.dma_start(cs2_sb[:], cs2_d[:])
        nc.sync.dma_start(ones_sb[:], ones_d[:])
        load_xh(0, 1, nc.scalar)
        # wo only needed at first outproj (~90us in)
        nc.scalar.dma_start(wo_sb[:], wo_d[:])

        # deferred output projection: [g0, ib, next-tile-cursor] or None
        pend = [None]

        def emit_yp(k=None, alt=False):
            # one outproj psum tile of the pending i-block (k overrides cursor)
            if pend[0] is None:
                return
            g0, ib, cur = pend[0]
            if k is None:
                k = cur
                pend[0] = None if cur + 1 == NYT else [g0, ib, cur + 1]
            tl, db = k // (D // IBLK), k % (D // IBLK)
            tt = ib * (IBLK // P) + tl
            yp = yps.tile([P, IBLK], f32, tag="y")
            for h in range(HPC):
                nc.tensor.matmul(
                    yp[:],
                    oT_sb[:, h * T + tt * P:h * T + (tt + 1) * P],
                    wo_sb[:, h * D + db * IBLK:h * D + (db + 1) * IBLK],
                    start=(h == 0), stop=(h == HPC - 1))
            yt = ypool.tile([P, IBLK], bf16, tag="yt")
            # steady state: scalar takes 2/8 (exp-limited); final drain:
            # alternate so the copies pipeline two wide
            on_scalar = (k % 2 == 0) if alt else (k % 8 < 2)
            if on_scalar:
                nc.scalar.copy(yt[:], yp[:])
            else:
                nc.vector.tensor_scalar_mul(yt[:], yp[:], 1.0)
            nc.sync.dma_start(
                y_d[g0 + tt * P:g0 + (tt + 1) * P,
                    db * IBLK:(db + 1) * IBLK],
                yt[:])

        for b in range(B):
            g0 = b * T

            with tc.tile_pool(name=f"qk_ps{b}", bufs=4, space="PSUM") as qkps, \
                 tc.tile_pool(name=f"v_ps{b}", bufs=1, space="PSUM") as vps:
                for blk in range(NBLK):
                    t0 = blk * TBLK
                    xt = xtiles[(b, blk // 2)]
                    xc0 = (blk % 2) * TBLK  # column offset inside the hb tile

                    for h in range(HPC):
                        for w_sb, dst in ((wq_sb, qT_sb), (wk_sb, kT_sb)):
                            pps = qkps.tile([P, TBLK], f32, tag="qk")
                            for dc in range(DC):
                                nc.tensor.matmul(
                                    pps[:],
                                    w_sb[:, dc * CPC + h * HD:dc * CPC + (h + 1) * HD],
                                    xt[:, dc * HB + xc0:dc * HB + xc0 + TBLK],
                                    start=(dc == 0), stop=(dc == DC - 1))
                            m1 = rpool.tile([P, TBLK], f32, tag="m1")
                            m3 = rpool.tile([P, TBLK], f32, tag="m3")
                            c1 = cs1_sb[:, t0:t0 + TBLK]
                            c2 = cs2_sb[:, t0:t0 + TBLK]
                            nc.vector.tensor_tensor(m1[:], pps[:], c1, mult)
                            nc.vector.tensor_tensor(m3[:], pps[:], c2, mult)
                            sw = rpool.tile([P, TBLK], f32, tag="sw")
                            nc.sync.dma_start(sw[0:64, :], m1[64:128, :])
                            nc.sync.dma_start(sw[64:128, :], m3[0:64, :])
                            o = dst[:, h * T + t0:h * T + t0 + TBLK]
                            nc.vector.tensor_tensor(
                                o[0:64, :], m1[0:64, :], sw[0:64, :], sub)
                            nc.vector.tensor_tensor(
                                o[64:128, :], m3[64:128, :], sw[64:128, :], add)
                            emit_yp()  # prev batch's last outproj block

                    vp = vps.tile([P, 4 * CPC], f32, tag="v")
                    for tl in range(TBLK // P):
                        for dc in range(DC):
                            nc.tensor.matmul(
                                vp[:, tl * CPC:(tl + 1) * CPC],
                                xt[:, dc * HB + xc0 + tl * P:dc * HB + xc0 + (tl + 1) * P],
                                wv_sb[:, dc * CPC:(dc + 1) * CPC],
                                start=(dc == 0), stop=(dc == DC - 1))
                    nc.scalar.copy(
                        v_sb[:, blk * 4 * CPC:(blk + 1) * 4 * CPC], vp[:])

            # prefetch next batch's x during this batch's attention
            if b + 1 < B:
                load_xh(b + 1, 0)
                load_xh(b + 1, 1)

            with tc.tile_pool(name=f"s_ps{b}", bufs=2, space="PSUM") as sps, \
                 tc.tile_pool(name=f"o_ps{b}", bufs=2, space="PSUM") as ops:
                for ib in range(NIB):
                    i0 = ib * IBLK
                    for h in range(HPC):
                        q_sl = qT_sb[:, h * T + i0:h * T + i0 + IBLK]
                        op = ops.tile([P, IBLK], f32, tag="o")
                        accA = accpool.tile([P, 2 * IBLK], bf16, tag="accA")
                        for pr in range(NPR):
                            j0, j1 = 2 * pr, 2 * pr + 1
                            sp = sps.tile([P, 2 * IBLK], f32, tag="s")
                            nc.tensor.matmul(
                                sp[:, 0:IBLK],
                                kT_sb[:, h * T + j0 * P:h * T + (j0 + 1) * P],
                                q_sl, start=True, stop=True)
                            nc.tensor.matmul(
                                sp[:, IBLK:2 * IBLK],
                                kT_sb[:, h * T + j1 * P:h * T + (j1 + 1) * P],
                                q_sl, start=True, stop=True)
                            e = epool.tile([P, 2 * IBLK], bf16, tag="e")
                            nc.scalar.activation(e[:], sp[:], Exp, scale=SCALE)
                            nc.tensor.matmul(
                                op[:],
                                v_sb[:, j0 * CPC + h * HD:j0 * CPC + (h + 1) * HD],
                                e[:, 0:IBLK], start=(pr == 0), stop=False)
                            nc.tensor.matmul(
                                op[:],
                                v_sb[:, j1 * CPC + h * HD:j1 * CPC + (h + 1) * HD],
                                e[:, IBLK:2 * IBLK],
                                start=False, stop=(pr == NPR - 1))
                            # denominator partial sum on DVE (bf16 2x mode;
                            # gpsimd measures ~7x slower than its cost model)
                            if pr == 0:
                                nc.vector.tensor_scalar_mul(accA[:], e[:], 1.0)
                            else:
                                nc.vector.tensor_tensor(accA[:], accA[:], e[:], add)
                            emit_yp()  # interleave prev i-block's outproj
                        # dn borrows a slot in the outproj psum rotation; it
                        # lives ~1us per (h,ib), freeing a bank for op bufs=2
                        dn = yps.tile([P, IBLK], f32, tag="y")
                        nc.tensor.matmul(dn[:], ones_sb[:], accA[:, 0:IBLK],
                                         start=True, stop=False)
                        nc.tensor.matmul(dn[:], ones_sb[:], accA[:, IBLK:2 * IBLK],
                                         start=False, stop=True)
                        rcp = rcpool.tile([P, IBLK], f32, tag="rc")
                        nc.vector.reciprocal_approx_fast(out=rcp[:], in_=dn[:])
                        nc.vector.tensor_tensor(
                            oT_sb[:, h * T + i0:h * T + i0 + IBLK],
                            op[:], rcp[:], mult)
                    while pend[0] is not None:  # drain any leftovers
                        emit_yp()
                    pend[0] = [g0, ib, 0]

        for k in range(NYT):  # final i-block's projection (tail)
            emit_yp(k, alt=True)


_EVEN_ODD = np.concatenate([np.arange(0, HD, 2), np.arange(1, HD, 2)])
_BF16 = ml_dtypes.bfloat16


def _pack_w(w):
    # [D, CPC] -> SBUF image [P, DC*CPC] (row p holds dc-major chunks)
    return np.ascontiguousarray(
        w.reshape(DC, P, CPC).transpose(1, 0, 2).reshape(P, DC * CPC)
        .astype(_BF16))


def _prep_inputs(x, rope_cos, rope_sin, Wq, Wk, Wv, Wo):
    x = np.asarray(x, dtype=np.float32)
    xT = x.reshape(BT, D).T  # [D, BT]
    # pack into per-half-batch SBUF images: row hb*P+p, col dc*HB+t
    xpk = np.ascontiguousarray(
        xT.reshape(DC, P, BT // HB, HB).transpose(2, 1, 0, 3)
        .reshape((BT // HB) * P, DC * HB).astype(_BF16))
    cosT = np.asarray(rope_cos, dtype=np.float32).T
    sinT = np.asarray(rope_sin, dtype=np.float32).T
    cs1 = np.ascontiguousarray(
        np.concatenate([cosT, sinT], axis=0), dtype=np.float32)
    cs2 = np.ascontiguousarray(
        np.concatenate([sinT, cosT], axis=0), dtype=np.float32)
    ones = np.ones((P, P), dtype=_BF16)
    Wq = np.asarray(Wq, dtype=np.float32)
    Wk = np.asarray(Wk, dtype=np.float32)
    Wv = np.asarray(Wv, dtype=np.float32)
    Wo = np.asarray(Wo, dtype=np.float32)

    in_maps = []
    for c in range(NCORES):
        cols = slice(c * CPC, (c + 1) * CPC)
        wq_c = Wq[:, cols].reshape(D, HPC, HD)[:, :, _EVEN_ODD].reshape(D, CPC)
        wk_c = Wk[:, cols].reshape(D, HPC, HD)[:, :, _EVEN_ODD].reshape(D, CPC)
        wo_c = Wo[cols, :]  # [CPC, D] -> [P, HPC*D]
        in_maps.append({
            "xT": xpk,
            "wq": _pack_w(wq_c),
            "wk": _pack_w(wk_c),
            "wv": _pack_w(Wv[:, cols]),
            "wo": np.ascontiguousarray(
                wo_c.reshape(HPC, P, D).transpose(1, 0, 2)
                .reshape(P, HPC * D).astype(_BF16)),
            "cs1": cs1,
            "cs2": cs2,
            "ones": ones,
        })
    return in_maps


def kernel(x, rope_cos, rope_sin, Wq, Wk, Wv, Wo, _trace=False):
    global last_results
    if "nc" not in _compiled:
        _compiled["nc"] = _build()
    nc = _compiled["nc"]
    in_maps = _prep_inputs(x, rope_cos, rope_sin, Wq, Wk, Wv, Wo)
    res = run_bass_kernel_spmd(
        nc, in_maps, core_ids=list(range(NCORES)), trace=_trace)
    last_results = res
    y = np.sum(np.stack([res.results[c]["y"].astype(np.float64)
                         for c in range(NCORES)]), axis=0)
    return y.reshape(B, T, D).astype(np.float32)
